# revision 92
# baseline (speedup 1.0000x reference)
"""AttnBlock2D (B=4, C=512, H=W=64) on 8 Trainium2 NeuronCores.

Strategy: data-parallel over batch x sequence-parallel over output tokens.
Core c handles image b = c//2 and output-token half h = c%2.  The host
permutes the token axis per core (own half first), so the SPMD program is
identical on every core.

Math (scores-transposed formulation, softmax axis j on partitions).  With
M = Wk^T Wq and u = Wq^T bk, the softmax-relevant part of the score is

    score[i,j] = x_i^T M x_j + u . x_j  = g_i . x_j,   g_i = M^T x_i + u

(bq and all i-only terms cancel in softmax).  So the kernel computes

    g           = M^T x + u            (GEMM over the core's i-half ONLY;
                                        the j-side operand is raw x)
    e^T[j, i]   = exp(SCALE * g_i . x_j)     (ScalarE, bf16 out)
    acc[p, i]   = sum_jc e^T[jc*128+p, i]    (DVE partial-sum, f32)
    s[i]        = 1^T @ acc                  (one matmul per i-block)
    u[c, i]     = sum_j vT'[j, c] e^T[j, i]  (lhsT = vT', rhs = e^T)
    y[c, i]     = u[c, i] / s[i]

with vT'[j, c] = ((Wo Wv) x_j)_c + (Wo bv + bo)_c: softmax weights sum to
one, so the whole output bias (and Wo itself) ride inside vT and the
epilogue is a single normalising multiply.  All matmul operands are
bfloat16 (full PE rate at any free dim, half the DMA/SBUF of f32);
accumulation stays f32.

Schedule: warm-up matmuls on a memset scratch absorb the PE p-state ramp
during the initial weight DMA; apply matmuls trail the scores by a few
j-chunks so each i-block's epilogue (sum/recip/broadcast/mul/store) hides
under the next block's score matmuls; final i-blocks narrow down
(512,512,512,256,128,128) so the last epilogue tail is short.
"""

import numpy as np

import concourse.bass as bass  # noqa: F401  (engine types via nc.*)
import concourse.tile as tile
import concourse.mybir as mybir
from concourse import bacc
from concourse.alu_op_type import AluOpType
from concourse.bass_utils import run_bass_kernel_spmd

B = 4
C = 512            # C_IN == C_HID
HW = 64 * 64       # tokens per image
NCORES = 8
I = HW * B // NCORES   # 2048 output tokens per core

CK = 128           # partition chunk
NB = 512           # x-tile token width
NCH = C // CK      # 4 channel chunks
NJB = HW // CK     # 32 j-chunks
NNB = HW // NB     # 8 x tiles
NIB_MINE = I // NB  # 4 x tiles in my half

BLOCKS = [512, 512, 512, 256, 128, 128]   # i-block widths (sum = I)
assert sum(BLOCKS) == I

F32 = mybir.dt.float32
F32R = mybir.dt.float32r
BF16 = mybir.dt.bfloat16
AF = mybir.ActivationFunctionType
SCALE = 1.0 / float(np.sqrt(float(C)))

N_WARMUP = 17      # dummy matmuls riding out the first weight DMA


def build_bass():
    nc = bacc.Bacc(
        "TRN2", target_bir_lowering=False, debug=False, enable_asserts=False
    )

    xb = nc.dram_tensor("xb", [C, HW], BF16, kind="ExternalInput").ap()
    wqT = nc.dram_tensor("wqT", [C, C], BF16, kind="ExternalInput").ap()
    wvT = nc.dram_tensor("wvT", [C, C], BF16, kind="ExternalInput").ap()
    ub = nc.dram_tensor("ub", [CK, NCH], F32, kind="ExternalInput").ap()
    borow = nc.dram_tensor("borow", [1, C], F32, kind="ExternalInput").ap()
    # out packed [p, i, a] so every store is per-partition contiguous (no
    # sub-512B descriptor penalty); host unpacks
    out = nc.dram_tensor("out", [CK, I, NCH], BF16,
                         kind="ExternalOutput").ap()

    x3 = xb.rearrange("(a p) n -> p a n", p=CK)     # [128, 4, 4096]
    wq3d = wqT.rearrange("(a p) n -> p a n", p=CK)  # [128, 4, 512]
    wv3d = wvT.rearrange("(a p) n -> p a n", p=CK)

    with tile.TileContext(nc) as tc:
        with tc.tile_pool(name="persist", bufs=1) as persist, \
             tc.tile_pool(name="ep", bufs=16) as ep, \
             tc.tile_pool(name="accp", bufs=2) as accp, \
             tc.tile_pool(name="rp", bufs=2) as rp, \
             tc.tile_pool(name="ftp", bufs=2) as ftp, \
             tc.tile_pool(name="psA", bufs=3, space="PSUM") as psA, \
             tc.tile_pool(name="psS", bufs=1, space="PSUM") as psS, \
             tc.tile_pool(name="psO", bufs=1, space="PSUM") as psO:

            # ---- persistent SBUF state ----
            x = persist.tile([CK, NCH, HW], BF16, name="x")   # all tokens
            g = persist.tile([CK, NCH, I], BF16, name="g")    # my i-half
            vT = persist.tile([CK, NJB, C], BF16, name="vT")
            ubias = persist.tile([CK, NCH], F32, name="ubias")
            bo_b = persist.tile([CK, C], F32, name="bo_b")
            bo_r = persist.tile([1, C], F32, name="bo_r")
            ones128 = persist.tile([CK, 1], F32, name="ones128")
            wq3 = persist.tile([CK, NCH, C], BF16, name="wq3")
            wv3 = persist.tile([CK, NCH, C], BF16, name="wv3")
            scratch = persist.tile([CK, CK], F32, name="scratch")

            # ---- input DMAs, first-needed first per queue ----
            # scalar queue: wq in ci-halves (nb=0's g groups start on the
            # first half while the second is still in flight), then odd x
            # tiles
            nc.scalar.dma_start(out=wq3[:, 0:2, :], in_=wq3d[:, 0:2, :])
            nc.scalar.dma_start(out=wq3[:, 2:4, :], in_=wq3d[:, 2:4, :])
            # sync queue: x tile nb=0 in ci-halves, then even tiles
            nc.sync.dma_start(out=x[:, 0:2, 0:NB], in_=x3[:, 0:2, 0:NB])
            nc.sync.dma_start(out=x[:, 2:4, 0:NB], in_=x3[:, 2:4, 0:NB])
            # gpsimd queue: scratch memset first (unblocks the PE warm-up
            # before the SWDGE descriptor generations), then small tensors,
            # then wv (phase B's v GEMMs)
            nc.gpsimd.memset(scratch, 0.0)
            nc.gpsimd.dma_start(out=ubias, in_=ub)
            nc.gpsimd.dma_start(out=bo_r, in_=borow)
            # x1 via SWDGE with its desc-gen after borow: the transfer
            # enters the shared-device queue after ALL four startup wq/x0
            # halves but well before nb=1's g GEMM needs it
            nc.gpsimd.dma_start(out=x[:, :, NB:2 * NB],
                                in_=x3[:, :, NB:2 * NB])
            # broadcast bo before the wv descriptor-gens: its borow wait
            # holds Pool.SEQ, so the wv transfers enter the shared DMA
            # device after the x1/x2 tiles the g phase needs (wv itself is
            # not needed until the v phase, ~18us in)
            nc.gpsimd.partition_broadcast(bo_b, bo_r)
            for ci in range(NCH):
                nc.gpsimd.dma_start(out=wv3[:, ci, :], in_=wv3d[:, ci, :])
            for nb in range(2, NNB):
                eng = nc.sync if nb % 2 == 0 else nc.scalar
                eng.dma_start(out=x[:, :, nb * NB:(nb + 1) * NB],
                              in_=x3[:, :, nb * NB:(nb + 1) * NB])

            # ---- warm-up: keep PE busy (and the p-state ramp burning)
            # while the first weight/x DMAs land; results land in a PSUM
            # bank that the first real accumulation group overwrites ----
            nc.vector.memset(ones128, 1.0)
            psD = psS.tile([CK, NB], F32, name="psD", tag="sum",
                           space="PSUM")
            scr_r = scratch.bitcast(F32R)
            for i in range(N_WARMUP):
                # one accumulation group: no inter-matmul semaphores
                nc.tensor.matmul(psD[:, 0:CK], lhsT=scr_r, rhs=scr_r,
                                 start=(i == 0), stop=(i == N_WARMUP - 1))

            # ---- phase B: g over my half first (needs only wq + x0..3,
            # matching DMA arrival order), then vT' for all tokens ----
            # nb=0 runs as two ci-half passes across all cc (in the idle
            # psO banks) so its matmuls start on the first half-DMAs
            pg0 = [psO.tile([CK, NB], F32, name=f"pg{cc}", tag=f"po{cc}",
                            space="PSUM") for cc in range(NCH)]
            for ci in range(NCH):
                for cc in range(NCH):
                    nc.tensor.matmul(
                        pg0[cc],
                        lhsT=wq3[:, ci, cc * CK:(cc + 1) * CK],
                        rhs=x[:, ci, 0:NB],
                        start=(ci == 0), stop=(ci == NCH - 1),
                    )
            for cc in range(NCH):
                nc.scalar.activation(
                    g[:, cc, 0:NB], pg0[cc], AF.Identity,
                    bias=ubias[:, cc:cc + 1])
            for nb in range(1, NIB_MINE):
                for cc in range(NCH):
                    pg = psA.tile([CK, NB], F32, name="pg", tag="psA",
                                  space="PSUM")
                    for ci in range(NCH):
                        nc.tensor.matmul(
                            pg,
                            lhsT=wq3[:, ci, cc * CK:(cc + 1) * CK],
                            rhs=x[:, ci, nb * NB:(nb + 1) * NB],
                            start=(ci == 0), stop=(ci == NCH - 1),
                        )
                    # g = M^T x + u; bq cancels in softmax
                    nc.scalar.activation(
                        g[:, cc, nb * NB:(nb + 1) * NB], pg, AF.Identity,
                        bias=ubias[:, cc:cc + 1])
            for jc in range(NJB):
                pv = psA.tile([CK, C], F32, name="pv", tag="psA",
                              space="PSUM")
                for ci in range(NCH):
                    nc.tensor.matmul(
                        pv,
                        lhsT=x[:, ci, jc * CK:(jc + 1) * CK],
                        rhs=wv3[:, ci, :],
                        start=(ci == 0), stop=(ci == NCH - 1),
                    )
                # vT' = (WoWv)x + (Wo bv + bo): softmax weights sum to
                # 1, so the output bias rides inside vT
                nc.vector.tensor_tensor(out=vT[:, jc, :], in0=pv,
                                        in1=bo_b, op=AluOpType.add)

            # ---- phase C: attention per i-block, software-pipelined ----
            offs = np.cumsum([0] + BLOCKS).tolist()

            state: dict = {}
            prev: dict = {}
            pending: list = []    # deferred closures from the previous block

            def scores_chunk(jc):
                w, off = state["w"], state["off"]
                ps_ = psA.tile([CK, w], F32, name="ps", tag="psA",
                               space="PSUM")
                for ci in range(NCH):
                    nc.tensor.matmul(
                        ps_,
                        lhsT=x[:, ci, jc * CK:(jc + 1) * CK],
                        rhs=g[:, ci, off:off + w],
                        start=(ci == 0), stop=(ci == NCH - 1),
                    )
                et = ep.tile([CK, w], BF16, name="et", tag="et")
                nc.scalar.activation(et, ps_, AF.Exp, scale=SCALE)
                acc = state["acc"][:, 0:w]
                if jc == 0:
                    nc.vector.tensor_copy(acc, et)
                else:
                    # alternate engines: DVE also runs the epilogue muls
                    # (gpsimd cannot touch PSUM), so share the adds
                    eng = nc.vector if jc % 2 == 0 else nc.gpsimd
                    eng.tensor_add(acc, acc, et)
                state["ets"][jc] = et

            def make_apply(st, jc):
                def go():
                    et = st["ets"][jc]
                    for cc in range(NCH):
                        nc.tensor.matmul(
                            st["po"][cc],
                            lhsT=vT[:, jc, cc * CK:(cc + 1) * CK],
                            rhs=et,
                            start=(jc == 0), stop=(jc == NJB - 1),
                        )
                return go

            def make_sum_chain(st):
                def go():
                    w = st["w"]
                    psum = psS.tile([1, w], F32, name="psum", tag="sum",
                                    space="PSUM")
                    nc.tensor.matmul(psum, lhsT=ones128.bitcast(F32R),
                                     rhs=st["acc"][:, 0:w],
                                     start=True, stop=True)
                    r1 = rp.tile([1, NB], F32, name="r1", tag="r1")
                    nc.vector.reciprocal(r1[:, 0:w], psum)
                    rb = rp.tile([CK, NB], F32, name="rb", tag="rb")
                    nc.gpsimd.partition_broadcast(rb[:, 0:w], r1[:, 0:w])
                    st["rb"] = rb
                return go

            def make_epilogue(st):
                def go():
                    w, off, rb = st["w"], st["off"], st["rb"]
                    ft = ftp.tile([CK, NB, NCH], BF16, name="ft", tag="ft")
                    for cc in range(NCH):
                        # must be DVE: gpsimd cannot read PSUM
                        nc.vector.tensor_tensor(out=ft[:, 0:w, cc],
                                                in0=st["po"][cc],
                                                in1=rb[:, 0:w],
                                                op=AluOpType.mult)
                    # one fused store: a single HWDGE generation on the
                    # shared DGE device instead of four
                    nc.sync.dma_start(out=out[:, off:off + w, :],
                                      in_=ft[:, 0:w, :])
                return go

            # per-width pipeline knobs: narrow blocks have shorter score
            # chunks, so the fixed-latency epilogue/exp chains need more
            # slots of cover before the applies start
            KNOBS = {512: (6, 3), 256: (9, 5), 128: (14, 9)}
            for k, w in enumerate(BLOCKS):
                start_slot, lag = KNOBS[w]
                state = {
                    "w": w,
                    "off": offs[k],
                    "acc": accp.tile([CK, NB], F32R, name="acc", tag="acc"),
                    "po": [psO.tile([CK, w], F32, name=f"po{cc}",
                                    tag=f"po{cc}", space="PSUM")
                           for cc in range(NCH)],
                    "ets": [None] * NJB,
                }
                nxt = 0          # next own apply chunk to emit
                for jc in range(NJB):
                    scores_chunk(jc)
                    if pending:
                        pending.pop(0)()
                        if not pending and prev:
                            # prev block's po complete: sum/recip/bcast were
                            # emitted at slot 1; normalising muls go now so
                            # they run under the scores-only slots below
                            make_epilogue(prev)()
                    elif jc >= start_slot:
                        # catch up two per slot, then settle at lag
                        for _ in range(2):
                            if nxt <= jc - lag:
                                make_apply(state, nxt)()
                                nxt += 1
                    if jc == 1 and prev:
                        make_sum_chain(prev)()
                # defer the last `lag` apply chunks into the next block
                pending = [make_apply(state, t) for t in range(nxt, NJB)]
                prev = state

            # tail: drain the last block.  The sum/recip/broadcast chain
            # only needs acc (complete shortly after the last exp), so emit
            # it after a few applies — it then overlaps the remaining apply
            # matmuls instead of serialising after them.
            for fn in pending[:5]:
                fn()
            make_sum_chain(prev)()
            for fn in pending[5:]:
                fn()
            make_epilogue(prev)()

    nc.compile()
    return nc


_NC = None


def _get_nc():
    global _NC
    if _NC is None:
        _NC = build_bass()
    return _NC


def _make_in_maps(inp, Wk, bk, Wq, bq, Wv, bv, Wo, bo):
    import ml_dtypes

    bf = ml_dtypes.bfloat16
    x_all = np.asarray(inp, dtype=np.float32).reshape(B, C, HW)
    # M = Wk^T Wq: lhsT of the g GEMM (g = M^T x + u)
    wqT = np.ascontiguousarray(
        (np.asarray(Wk, np.float64).T @ np.asarray(Wq, np.float64))
    ).astype(bf)
    wvT = np.ascontiguousarray(
        (np.asarray(Wo, np.float64) @ np.asarray(Wv, np.float64)).T
    ).astype(bf)
    u = (np.asarray(Wq, np.float64).T @ np.asarray(bk, np.float64))
    ub = np.ascontiguousarray(
        u.astype(np.float32).reshape(NCH, CK).T)
    bo_eff = (np.asarray(Wo, np.float32) @ np.asarray(bv, np.float32)
              + np.asarray(bo, np.float32))
    borow = np.ascontiguousarray(bo_eff.reshape(1, C).astype(np.float32))

    in_maps = []
    for c in range(NCORES):
        b, h = divmod(c, NCORES // B)
        xb = x_all[b]
        if h == 1:      # own half first; attention is j-permutation-invariant
            xb = np.concatenate([xb[:, I:], xb[:, :I]], axis=1)
        in_maps.append({
            "xb": np.ascontiguousarray(xb).astype(bf),
            "wqT": wqT, "wvT": wvT, "ub": ub, "borow": borow,
        })
    return in_maps


def run(trace=False, tmpdir=None, **inputs):
    nc = _get_nc()
    in_maps = _make_in_maps(**inputs)
    res = run_bass_kernel_spmd(
        nc, in_maps, core_ids=list(range(NCORES)), trace=trace, tmpdir=tmpdir
    )
    full = np.empty((B, C, HW), dtype=np.float32)
    for c in range(NCORES):
        b, h = divmod(c, NCORES // B)
        # device layout [p, i, a] -> channel c = a*CK + p
        o = np.asarray(res.results[c]["out"]).astype(np.float32)
        full[b][:, h * I:(h + 1) * I] = o.transpose(2, 0, 1).reshape(C, I)
    return full.reshape(B, C, 64, 64), res


def kernel(**inputs):
    out, _ = run(trace=False, **inputs)
    return out
